# revision 11
# baseline (speedup 1.0000x reference)
"""Trainium2 Bass kernel for nn_CalibrationNetwork (MoE routing over 12 judges).

Strategy: shared + judge-specific weights are pre-summed on the host into 12
effective per-judge MLPs (the einsum+take_along_axis in the reference is just
"route each sample through the MLP of its judge").  Samples are sorted by
judge id on the host, each judge's slots padded to a fixed capacity 2*Cc, and
the resulting 24 fixed-size chunks (2 per judge) are dealt 3-per-core to the 8
NeuronCores.  Every core runs the same static Bass/Tile program: for each of
its 3 chunks, dense matmuls (layer1 K=36, layer2 K=256, heads K=256) with
relu/bias fused into the PSUM-evacuation.  The 7x5 per-question softmax runs
in head-major layout: exp(logits+bias) via the ACT engine's per-partition
bias, group sums and the reciprocal broadcast via exact block-ones matmuls on
the PE.  Output is written head-major (contiguous rows) and transposed back
on the host during the unshard scatter.
"""

import os
import sys

import numpy as np

for _p in ("/opt/trn_rl_repo", "/root/.axon_site/_ro/trn_rl_repo"):
    if os.path.isdir(_p) and _p not in sys.path:
        sys.path.insert(0, _p)

B, D, H1, H2, J, Q, O = 32768, 35, 256, 256, 12, 7, 5
NCORES = 8
SEG = 3                    # chunks per core
NCHUNKS = NCORES * SEG     # 24 = 2 chunks per judge
QO = Q * O                 # 35
QOp = QO + 1               # padded head dim (f32r needs even sizes)
Qp = Q + 1                 # padded question dim

USE_F32R = True            # PE fast-fp32 mode (1 cyc/row vs 4 for fp32)
TRACE = False              # set True in test harness to collect NTFF profile
LAST_RESULTS = None        # BassKernelResults of the last run (for test.py)

_PROG_CACHE = {}


def _build_program(Cc, use_f32r):
    import concourse.bass as bass
    import concourse.tile as tile
    from concourse import bacc, mybir

    f32 = mybir.dt.float32
    fmm = mybir.dt.float32r if use_f32r else f32
    AF = mybir.ActivationFunctionType
    ALU = mybir.AluOpType

    NT = Cc // 512            # 512-wide n-tiles per chunk

    nc = bacc.Bacc(None, target_bir_lowering=False, debug=False)

    xt_d = nc.dram_tensor("xt", [D + 1, SEG * Cc], fmm, kind="ExternalInput")
    a1_d = nc.dram_tensor("a1t", [SEG, D + 1, H1], fmm, kind="ExternalInput")
    a2_d = nc.dram_tensor("a2t", [SEG, 128, 2, H2], fmm, kind="ExternalInput")
    b2_d = nc.dram_tensor("b2", [SEG, 128, 2], f32, kind="ExternalInput")
    av_d = nc.dram_tensor("avt", [SEG, 128, 2, QOp], fmm, kind="ExternalInput")
    bv_d = nc.dram_tensor("bv", [SEG, QOp], f32, kind="ExternalInput")
    ones_s_d = nc.dram_tensor("ones_s", [QOp, Qp], fmm, kind="ExternalInput")
    ones_r_d = nc.dram_tensor("ones_r", [Qp, QOp], fmm, kind="ExternalInput")
    out_d = nc.dram_tensor("out", [QO, SEG * Cc], f32, kind="ExternalOutput")

    import contextlib

    lp = (
        nc.allow_low_precision(reason="float32r matmul operands are intentional")
        if use_f32r
        else contextlib.nullcontext()
    )
    with lp, tile.TileContext(nc) as tc:
        with (
            tc.tile_pool(name="xp", bufs=1) as xp,
            tc.tile_pool(name="wp", bufs=2) as wp,
            tc.tile_pool(name="zp", bufs=2) as zp,
            tc.tile_pool(name="op", bufs=3) as op_,
            tc.tile_pool(name="psA", bufs=2, space="PSUM") as psA,
            tc.tile_pool(name="psB", bufs=1, space="PSUM") as psB,
        ):
            xt = xp.tile([D + 1, SEG * Cc], fmm)
            nc.sync.dma_start(xt[:], xt_d[:])
            ones_s = xp.tile([QOp, Qp], fmm)
            nc.sync.dma_start(ones_s[:], ones_s_d[:])
            ones_r = xp.tile([Qp, QOp], fmm)
            nc.sync.dma_start(ones_r[:], ones_r_d[:])

            for s in range(SEG):
                a1 = wp.tile([D + 1, H1], fmm, tag="a1")
                nc.sync.dma_start(a1[:], a1_d[s])
                a2 = wp.tile([128, 2, H2], fmm, tag="a2")
                nc.sync.dma_start(a2[:], a2_d[s])
                b2 = wp.tile([128, 2], f32, tag="b2")
                nc.sync.dma_start(b2[:], b2_d[s])
                av = wp.tile([128, 2, QOp], fmm, tag="av")
                nc.sync.dma_start(av[:], av_d[s])
                bv = wp.tile([QOp, 1], f32, tag="bv")
                nc.sync.dma_start(bv[:], bv_d[s][:, None])

                z1 = zp.tile([128, 2, Cc], fmm, tag="z1")
                z2 = zp.tile([128, 2, Cc], fmm, tag="z2")

                # ---- layer 1: z1 = relu(xb @ A1eff.T), bias folded in ones col
                for m in range(2):
                    for n in range(NT):
                        p1 = psA.tile([128, 512], f32, tag="l1")
                        nc.tensor.matmul(
                            p1[:],
                            a1[:, m * 128 : (m + 1) * 128],
                            xt[:, s * Cc + n * 512 : s * Cc + (n + 1) * 512],
                            start=True,
                            stop=True,
                        )
                        dst = z1[:, m, n * 512 : (n + 1) * 512]
                        if n % 2 == 0:
                            nc.scalar.activation(dst, p1[:], AF.Relu)
                        else:
                            nc.vector.tensor_scalar(
                                out=dst, in0=p1[:], scalar1=0.0, scalar2=None,
                                op0=ALU.max,
                            )

                # ---- layer 2: z2 = relu(z1b @ A2eff.T + b2)
                for m in range(2):
                    for n in range(NT):
                        p2 = psA.tile([128, 512], f32, tag="l2")
                        for k in range(2):
                            nc.tensor.matmul(
                                p2[:],
                                a2[:, k, m * 128 : (m + 1) * 128],
                                z1[:, k, n * 512 : (n + 1) * 512],
                                start=(k == 0),
                                stop=(k == 1),
                            )
                        dst = z2[:, m, n * 512 : (n + 1) * 512]
                        if n % 2 == 1:
                            nc.scalar.activation(
                                dst, p2[:], AF.Relu, bias=b2[:, m : m + 1]
                            )
                        else:
                            nc.vector.tensor_scalar(
                                out=dst, in0=p2[:],
                                scalar1=b2[:, m : m + 1], scalar2=0.0,
                                op0=ALU.add, op1=ALU.max,
                            )

                # ---- heads + grouped softmax, head-major (QOp x 512) tiles
                for n in range(NT):
                    nsl = slice(n * 512, (n + 1) * 512)
                    ph = psA.tile([QOp, 512], f32, tag="hd")
                    for k in range(2):
                        nc.tensor.matmul(
                            ph[:],
                            av[:, k, :],
                            z2[:, k, nsl],
                            start=(k == 0),
                            stop=(k == 1),
                        )
                    # e = exp(logits + bias); pad row 35 gets bias -1e30 -> 0
                    e = op_.tile([QOp, 512], fmm, tag="e")
                    nc.scalar.activation(e[:], ph[:], AF.Exp, bias=bv[:])
                    # group sums (row Q holds total, keeps reciprocal finite)
                    sm = psB.tile([Qp, 512], f32, tag="sm")
                    nc.tensor.matmul(sm[:], ones_s[:], e[:], start=True, stop=True)
                    rt = op_.tile([Qp, 512], fmm, tag="rt")
                    nc.vector.reciprocal(rt[:], sm[:])
                    # broadcast reciprocal back to the 35 head rows
                    rb = psB.tile([QOp, 512], f32, tag="rb")
                    nc.tensor.matmul(rb[:], ones_r[:], rt[:], start=True, stop=True)
                    outm = op_.tile([QO, 512], f32, tag="outm")
                    nc.vector.tensor_tensor(outm[:], e[:QO, :], rb[:QO, :], ALU.mult)
                    nc.sync.dma_start(out_d[:, s * Cc + n * 512 : s * Cc + (n + 1) * 512], outm[:])

    nc.compile()
    return nc


def _get_program(Cc, use_f32r):
    key = (Cc, use_f32r)
    if key not in _PROG_CACHE:
        _PROG_CACHE[key] = _build_program(Cc, use_f32r)
    return _PROG_CACHE[key]


def kernel(**inputs):
    global LAST_RESULTS
    x = np.ascontiguousarray(np.asarray(inputs["x"], dtype=np.float32))
    ids = np.asarray(inputs["judge_ids"]).astype(np.int64).ravel()
    W1_w = np.asarray(inputs["W1_w"], np.float32)
    W1_b = np.asarray(inputs["W1_b"], np.float32)
    W2_w = np.asarray(inputs["W2_w"], np.float32)
    W2_b = np.asarray(inputs["W2_b"], np.float32)
    W1a_w = np.asarray(inputs["W1a_w"], np.float32)
    W1a_b = np.asarray(inputs["W1a_b"], np.float32)
    W2a_w = np.asarray(inputs["W2a_w"], np.float32)
    W2a_b = np.asarray(inputs["W2a_b"], np.float32)
    V_w = np.asarray(inputs["V_w"], np.float32)
    V_b = np.asarray(inputs["V_b"], np.float32)
    Va_w = np.asarray(inputs["Va_w"], np.float32)
    Va_b = np.asarray(inputs["Va_b"], np.float32)

    Bx = x.shape[0]
    cnts = np.bincount(ids, minlength=J)
    Cc = 1536
    mx = int(cnts.max())
    if 2 * Cc < mx:
        Cc = ((mx + 1) // 2 + 511) // 512 * 512

    # effective per-judge weights (shared + judge-specific, biases folded)
    A1 = (W1_w[None] + W1a_w).copy()                      # (J, H1, D+1)
    A1[:, :, D] += W1_b[None] + W1a_b
    A2 = W2_w[None] + W2a_w                               # (J, H2, H1+1)
    b2 = A2[:, :, H1] + W2_b[None] + W2a_b                # (J, H2)
    A2c = A2[:, :, :H1]                                   # (J, H2, H1)
    AV = (V_w[None] + Va_w).reshape(J, QO, H2 + 1)
    bV = (AV[:, :, H2] + (V_b[None] + Va_b).reshape(J, QO)).astype(np.float32)
    AVc = AV[:, :, :H2]

    # SBUF layouts
    a1sb = np.ascontiguousarray(np.transpose(A1, (0, 2, 1)))  # (J, 36, 256)
    a2sb = np.ascontiguousarray(
        np.transpose(A2c.reshape(J, H2, 2, 128), (0, 3, 2, 1))
    )  # (J, 128, 2, 256): [j,p,k,m] = A2c[j][m, k*128+p]
    b2sb = np.ascontiguousarray(np.transpose(b2.reshape(J, 2, 128), (0, 2, 1)))
    avsb = np.transpose(AVc.reshape(J, QO, 2, 128), (0, 3, 2, 1))  # (J,128,2,35)
    avsb = np.concatenate(
        [avsb, np.zeros((J, 128, 2, 1), np.float32)], axis=3
    )  # pad head out dim to 36
    avsb = np.ascontiguousarray(avsb)
    bvp = np.concatenate(
        [bV, np.full((J, 1), -1e30, np.float32)], axis=1
    )  # (J, 36): pad row bias -> exp = 0

    # block-ones matrices for the softmax group sum / reciprocal broadcast
    ones_s = np.zeros((QOp, Qp), np.float32)
    for o_ in range(QO):
        ones_s[o_, o_ // O] = 1.0
    ones_s[:, Q] = 1.0          # row Q of sums = total (keeps recip finite)
    ones_r = np.zeros((Qp, QOp), np.float32)
    for o_ in range(QO):
        ones_r[o_ // O, o_] = 1.0

    # slot -> sample map: judge j owns slots [j*2Cc, (j+1)*2Cc)
    order = np.argsort(ids, kind="stable")
    slot2samp = np.full(NCHUNKS * Cc, -1, np.int64)
    pos = 0
    for j in range(J):
        k = int(cnts[j])
        slot2samp[j * 2 * Cc : j * 2 * Cc + k] = order[pos : pos + k]
        pos += k
    chunk_judge = np.repeat(np.arange(J), 2)

    in_maps = []
    core_meta = []
    for c in range(NCORES):
        sl = slot2samp[c * SEG * Cc : (c + 1) * SEG * Cc]
        valid = sl >= 0
        Xc = np.zeros((SEG * Cc, D + 1), np.float32)
        Xc[valid, :D] = x[sl[valid]]
        Xc[:, D] = 1.0
        js = chunk_judge[c * SEG : (c + 1) * SEG]
        in_maps.append(
            {
                "xt": np.ascontiguousarray(Xc.T),
                "a1t": np.ascontiguousarray(a1sb[js]),
                "a2t": np.ascontiguousarray(a2sb[js]),
                "b2": np.ascontiguousarray(b2sb[js]),
                "avt": np.ascontiguousarray(avsb[js]),
                "bv": np.ascontiguousarray(bvp[js]),
                "ones_s": ones_s,
                "ones_r": ones_r,
            }
        )
        core_meta.append((sl, valid))

    nc = _get_program(Cc, USE_F32R)
    from concourse.bass_utils import run_bass_kernel_spmd

    res = run_bass_kernel_spmd(
        nc,
        in_maps,
        core_ids=list(range(NCORES)),
        trace=TRACE,
    )
    LAST_RESULTS = res

    full = np.zeros((Bx, Q, O), np.float32)
    for c in range(NCORES):
        oc = np.asarray(res.results[c]["out"]).T    # (SEG*Cc, 35)
        sl, valid = core_meta[c]
        full[sl[valid]] = oc[valid].reshape(-1, Q, O)
    return full


# revision 16
# speedup vs baseline: 1.1403x; 1.1403x over previous
"""Trainium2 Bass kernel for nn_CalibrationNetwork (MoE routing over 12 judges).

Strategy: shared + judge-specific weights are pre-summed on the host into 12
effective per-judge MLPs (the einsum+take_along_axis in the reference is just
"route each sample through the MLP of its judge").  Samples are sorted by
judge id on the host, each judge's slots padded to a fixed capacity 2*Cc, and
the resulting 24 fixed-size chunks (2 per judge) are dealt 3-per-core to the 8
NeuronCores.  Every core runs the same static Bass/Tile program: for each of
its 3 chunks, dense matmuls (layer1 K=36, layer2 K=256, heads K=256) with
relu/bias fused into the PSUM-evacuation.  The 7x5 per-question softmax runs
in head-major layout: exp(logits+bias) via the ACT engine's per-partition
bias, group sums and the reciprocal broadcast via exact block-ones matmuls on
the PE.  Output is written head-major (contiguous rows) and transposed back
on the host during the unshard scatter.
"""

import os
import sys

import numpy as np

for _p in ("/opt/trn_rl_repo", "/root/.axon_site/_ro/trn_rl_repo"):
    if os.path.isdir(_p) and _p not in sys.path:
        sys.path.insert(0, _p)

B, D, H1, H2, J, Q, O = 32768, 35, 256, 256, 12, 7, 5
NCORES = 8
SEG = 3                    # chunks per core
NCHUNKS = NCORES * SEG     # 24 = 2 chunks per judge
QO = Q * O                 # 35
QOp = QO + 1               # padded head dim (f32r needs even sizes)
Qp = Q + 1                 # padded question dim

USE_F32R = True            # PE fast-fp32 mode (1 cyc/row vs 4 for fp32)
TRACE = False              # set True in test harness to collect NTFF profile
LAST_RESULTS = None        # BassKernelResults of the last run (for test.py)

_PROG_CACHE = {}


def _build_program(Cc, use_f32r):
    import concourse.bass as bass
    import concourse.tile as tile
    from concourse import bacc, mybir

    f32 = mybir.dt.float32
    fmm = mybir.dt.float32r if use_f32r else f32
    AF = mybir.ActivationFunctionType
    ALU = mybir.AluOpType

    NT = Cc // 512            # 512-wide n-tiles per chunk

    nc = bacc.Bacc(None, target_bir_lowering=False, debug=False)

    xt_d = nc.dram_tensor("xt", [D + 1, SEG * Cc], fmm, kind="ExternalInput")
    a1_d = nc.dram_tensor("a1t", [SEG, D + 1, H1], fmm, kind="ExternalInput")
    a2_d = nc.dram_tensor("a2t", [SEG, 128, 2, H2], fmm, kind="ExternalInput")
    b2_d = nc.dram_tensor("b2", [SEG, 128, 2], f32, kind="ExternalInput")
    av_d = nc.dram_tensor("avt", [SEG, 128, 2, QOp], fmm, kind="ExternalInput")
    bv_d = nc.dram_tensor("bv", [SEG, QOp], f32, kind="ExternalInput")
    ones_s_d = nc.dram_tensor("ones_s", [QOp, Qp], fmm, kind="ExternalInput")
    ones_r_d = nc.dram_tensor("ones_r", [Qp, QOp], fmm, kind="ExternalInput")
    out_d = nc.dram_tensor("out", [QO, SEG * Cc], f32, kind="ExternalOutput")

    import contextlib

    lp = (
        nc.allow_low_precision(reason="float32r matmul operands are intentional")
        if use_f32r
        else contextlib.nullcontext()
    )
    with lp, tile.TileContext(nc) as tc:
        with (
            tc.tile_pool(name="xp", bufs=1) as xp,
            tc.tile_pool(name="wp", bufs=2) as wp,
            tc.tile_pool(name="zp", bufs=2) as zp,
            tc.tile_pool(name="op", bufs=3) as op_,
            tc.tile_pool(name="psA", bufs=2, space="PSUM") as psA,
            tc.tile_pool(name="psB", bufs=2, space="PSUM") as psB,
        ):
            ones_s = xp.tile([QOp, Qp], fmm)
            nc.sync.dma_start(ones_s[:], ones_s_d[:])
            ones_r = xp.tile([Qp, QOp], fmm)
            nc.sync.dma_start(ones_r[:], ones_r_d[:])

            xts = []
            for s in range(SEG):
                xc = xp.tile([D + 1, Cc], fmm, tag=f"xc{s}")
                nc.sync.dma_start(xc[:], xt_d[:, s * Cc : (s + 1) * Cc])
                xts.append(xc)

            for s in range(SEG):
                xt = xts[s]
                a1 = wp.tile([D + 1, H1], fmm, tag="a1")
                nc.sync.dma_start(a1[:], a1_d[s])
                a2 = wp.tile([128, 2, H2], fmm, tag="a2")
                nc.sync.dma_start(a2[:], a2_d[s])
                b2 = wp.tile([128, 2], f32, tag="b2")
                nc.sync.dma_start(b2[:], b2_d[s])
                av = wp.tile([128, 2, QOp], fmm, tag="av")
                nc.sync.dma_start(av[:], av_d[s])
                bv = wp.tile([QOp, 1], f32, tag="bv")
                nc.sync.dma_start(bv[:], bv_d[s][:, None])

                z1 = zp.tile([128, 2, Cc], fmm, tag="z1")
                z2 = zp.tile([128, 2, Cc], fmm, tag="z2")

                # ---- layer 1: z1 = relu(xb @ A1eff.T), bias folded in ones col
                for m in range(2):
                    for n in range(NT):
                        p1 = psA.tile([128, 512], f32, tag="l1")
                        nc.tensor.matmul(
                            p1[:],
                            a1[:, m * 128 : (m + 1) * 128],
                            xt[:, n * 512 : (n + 1) * 512],
                            start=True,
                            stop=True,
                        )
                        dst = z1[:, m, n * 512 : (n + 1) * 512]
                        if n % 2 == 0:
                            nc.scalar.activation(dst, p1[:], AF.Relu)
                        else:
                            nc.vector.tensor_scalar(
                                out=dst, in0=p1[:], scalar1=0.0, scalar2=None,
                                op0=ALU.max,
                            )

                # ---- layer 2: z2 = relu(z1b @ A2eff.T + b2)
                for m in range(2):
                    for n in range(NT):
                        p2 = psA.tile([128, 512], f32, tag="l2")
                        for k in range(2):
                            nc.tensor.matmul(
                                p2[:],
                                a2[:, k, m * 128 : (m + 1) * 128],
                                z1[:, k, n * 512 : (n + 1) * 512],
                                start=(k == 0),
                                stop=(k == 1),
                            )
                        dst = z2[:, m, n * 512 : (n + 1) * 512]
                        if n % 2 == 1:
                            nc.scalar.activation(
                                dst, p2[:], AF.Relu, bias=b2[:, m : m + 1]
                            )
                        else:
                            nc.vector.tensor_scalar(
                                out=dst, in0=p2[:],
                                scalar1=b2[:, m : m + 1], scalar2=0.0,
                                op0=ALU.add, op1=ALU.max,
                            )

                # ---- heads + grouped softmax, head-major (QOp x 512) tiles
                for n in range(NT):
                    nsl = slice(n * 512, (n + 1) * 512)
                    ph = psA.tile([QOp, 512], f32, tag="hd")
                    for k in range(2):
                        nc.tensor.matmul(
                            ph[:],
                            av[:, k, :],
                            z2[:, k, nsl],
                            start=(k == 0),
                            stop=(k == 1),
                        )
                    # e = exp(logits + bias); pad row 35 gets bias -1e30 -> 0
                    e = op_.tile([QOp, 512], fmm, tag="e")
                    nc.scalar.activation(e[:], ph[:], AF.Exp, bias=bv[:])
                    # group sums into rows 0..6 of a shared PSUM bank; row Q
                    # holds the total so every row stays positive/finite
                    smrb = psB.tile([QOp, 512], f32, tag="smrb")
                    nc.tensor.matmul(
                        smrb[:Qp, :], ones_s[:], e[:], start=True, stop=True
                    )
                    rt_f = op_.tile([Qp, 512], f32, tag="rt_f")
                    nc.vector.reciprocal_approx_fast(rt_f[:], smrb[:Qp, :])
                    if use_f32r:
                        rt = op_.tile([Qp, 512], fmm, tag="rt")
                        nc.vector.tensor_copy(rt[:], rt_f[:])
                    else:
                        rt = rt_f
                    # broadcast reciprocal back to the 35 head rows
                    nc.tensor.matmul(smrb[:], ones_r[:], rt[:], start=True, stop=True)
                    outm = op_.tile([QO, 512], f32, tag="outm")
                    nc.vector.tensor_tensor(outm[:], e[:QO, :], smrb[:QO, :], ALU.mult)
                    nc.sync.dma_start(out_d[:, s * Cc + n * 512 : s * Cc + (n + 1) * 512], outm[:])

    nc.compile()
    return nc


def _get_program(Cc, use_f32r):
    key = (Cc, use_f32r)
    if key not in _PROG_CACHE:
        _PROG_CACHE[key] = _build_program(Cc, use_f32r)
    return _PROG_CACHE[key]


def kernel(**inputs):
    global LAST_RESULTS
    x = np.ascontiguousarray(np.asarray(inputs["x"], dtype=np.float32))
    ids = np.asarray(inputs["judge_ids"]).astype(np.int64).ravel()
    W1_w = np.asarray(inputs["W1_w"], np.float32)
    W1_b = np.asarray(inputs["W1_b"], np.float32)
    W2_w = np.asarray(inputs["W2_w"], np.float32)
    W2_b = np.asarray(inputs["W2_b"], np.float32)
    W1a_w = np.asarray(inputs["W1a_w"], np.float32)
    W1a_b = np.asarray(inputs["W1a_b"], np.float32)
    W2a_w = np.asarray(inputs["W2a_w"], np.float32)
    W2a_b = np.asarray(inputs["W2a_b"], np.float32)
    V_w = np.asarray(inputs["V_w"], np.float32)
    V_b = np.asarray(inputs["V_b"], np.float32)
    Va_w = np.asarray(inputs["Va_w"], np.float32)
    Va_b = np.asarray(inputs["Va_b"], np.float32)

    Bx = x.shape[0]
    cnts = np.bincount(ids, minlength=J)
    Cc = 1536
    mx = int(cnts.max())
    if 2 * Cc < mx:
        Cc = ((mx + 1) // 2 + 511) // 512 * 512

    # effective per-judge weights (shared + judge-specific, biases folded)
    A1 = (W1_w[None] + W1a_w).copy()                      # (J, H1, D+1)
    A1[:, :, D] += W1_b[None] + W1a_b
    A2 = W2_w[None] + W2a_w                               # (J, H2, H1+1)
    b2 = A2[:, :, H1] + W2_b[None] + W2a_b                # (J, H2)
    A2c = A2[:, :, :H1]                                   # (J, H2, H1)
    AV = (V_w[None] + Va_w).reshape(J, QO, H2 + 1)
    bV = (AV[:, :, H2] + (V_b[None] + Va_b).reshape(J, QO)).astype(np.float32)
    AVc = AV[:, :, :H2]

    # SBUF layouts
    a1sb = np.ascontiguousarray(np.transpose(A1, (0, 2, 1)))  # (J, 36, 256)
    a2sb = np.ascontiguousarray(
        np.transpose(A2c.reshape(J, H2, 2, 128), (0, 3, 2, 1))
    )  # (J, 128, 2, 256): [j,p,k,m] = A2c[j][m, k*128+p]
    b2sb = np.ascontiguousarray(np.transpose(b2.reshape(J, 2, 128), (0, 2, 1)))
    avsb = np.transpose(AVc.reshape(J, QO, 2, 128), (0, 3, 2, 1))  # (J,128,2,35)
    avsb = np.concatenate(
        [avsb, np.zeros((J, 128, 2, 1), np.float32)], axis=3
    )  # pad head out dim to 36
    avsb = np.ascontiguousarray(avsb)
    bvp = np.concatenate(
        [bV, np.full((J, 1), -1e30, np.float32)], axis=1
    )  # (J, 36): pad row bias -> exp = 0

    # block-ones matrices for the softmax group sum / reciprocal broadcast
    ones_s = np.zeros((QOp, Qp), np.float32)
    for o_ in range(QO):
        ones_s[o_, o_ // O] = 1.0
    ones_s[:, Q] = 1.0          # row Q of sums = total (keeps recip finite)
    ones_r = np.zeros((Qp, QOp), np.float32)
    for o_ in range(QO):
        ones_r[o_ // O, o_] = 1.0

    # slot -> sample map: judge j owns slots [j*2Cc, (j+1)*2Cc)
    order = np.argsort(ids, kind="stable")
    slot2samp = np.full(NCHUNKS * Cc, -1, np.int64)
    pos = 0
    for j in range(J):
        k = int(cnts[j])
        slot2samp[j * 2 * Cc : j * 2 * Cc + k] = order[pos : pos + k]
        pos += k
    chunk_judge = np.repeat(np.arange(J), 2)

    in_maps = []
    core_meta = []
    for c in range(NCORES):
        sl = slot2samp[c * SEG * Cc : (c + 1) * SEG * Cc]
        valid = sl >= 0
        Xc = np.zeros((SEG * Cc, D + 1), np.float32)
        Xc[valid, :D] = x[sl[valid]]
        Xc[:, D] = 1.0
        js = chunk_judge[c * SEG : (c + 1) * SEG]
        in_maps.append(
            {
                "xt": np.ascontiguousarray(Xc.T),
                "a1t": np.ascontiguousarray(a1sb[js]),
                "a2t": np.ascontiguousarray(a2sb[js]),
                "b2": np.ascontiguousarray(b2sb[js]),
                "avt": np.ascontiguousarray(avsb[js]),
                "bv": np.ascontiguousarray(bvp[js]),
                "ones_s": ones_s,
                "ones_r": ones_r,
            }
        )
        core_meta.append((sl, valid))

    nc = _get_program(Cc, USE_F32R)
    from concourse.bass_utils import run_bass_kernel_spmd

    res = run_bass_kernel_spmd(
        nc,
        in_maps,
        core_ids=list(range(NCORES)),
        trace=TRACE,
    )
    LAST_RESULTS = res

    full = np.zeros((Bx, Q, O), np.float32)
    for c in range(NCORES):
        oc = np.asarray(res.results[c]["out"]).T    # (SEG*Cc, 35)
        sl, valid = core_meta[c]
        full[sl[valid]] = oc[valid].reshape(-1, Q, O)
    return full


# revision 18
# speedup vs baseline: 1.1604x; 1.0177x over previous
"""Trainium2 Bass kernel for nn_CalibrationNetwork (MoE routing over 12 judges).

Strategy: shared + judge-specific weights are pre-summed on the host into 12
effective per-judge MLPs (the einsum+take_along_axis in the reference is just
"route each sample through the MLP of its judge").  Samples are sorted by
judge id on the host, each judge's slots padded to a fixed capacity 2*Cc, and
the resulting 24 fixed-size chunks (2 per judge) are dealt 3-per-core to the 8
NeuronCores.  Every core runs the same static Bass/Tile program: for each of
its 3 chunks, dense matmuls (layer1 K=36, layer2 K=256, heads K=256) with
relu/bias fused into the PSUM-evacuation.  The 7x5 per-question softmax runs
in head-major layout: exp(logits+bias) via the ACT engine's per-partition
bias, group sums and the reciprocal broadcast via exact block-ones matmuls on
the PE.  Output is written head-major (contiguous rows) and transposed back
on the host during the unshard scatter.
"""

import os
import sys

import numpy as np

for _p in ("/opt/trn_rl_repo", "/root/.axon_site/_ro/trn_rl_repo"):
    if os.path.isdir(_p) and _p not in sys.path:
        sys.path.insert(0, _p)

B, D, H1, H2, J, Q, O = 32768, 35, 256, 256, 12, 7, 5
NCORES = 8
SEG = 3                    # chunks per core
NCHUNKS = NCORES * SEG     # 24 = 2 chunks per judge
QO = Q * O                 # 35
QOp = QO + 1               # padded head dim (f32r needs even sizes)
Qp = Q + 1                 # padded question dim

USE_F32R = True            # PE fast-fp32 mode (1 cyc/row vs 4 for fp32)
TRACE = False              # set True in test harness to collect NTFF profile
LAST_RESULTS = None        # BassKernelResults of the last run (for test.py)

_PROG_CACHE = {}


def _build_program(Cc, use_f32r):
    import concourse.bass as bass
    import concourse.tile as tile
    from concourse import bacc, mybir

    f32 = mybir.dt.float32
    fmm = mybir.dt.float32r if use_f32r else f32
    AF = mybir.ActivationFunctionType
    ALU = mybir.AluOpType

    NT = Cc // 512            # 512-wide n-tiles per chunk

    nc = bacc.Bacc(None, target_bir_lowering=False, debug=False)

    xt_d = nc.dram_tensor("xt", [D + 1, SEG * Cc], fmm, kind="ExternalInput")
    a1_d = nc.dram_tensor("a1t", [SEG, D + 1, H1], fmm, kind="ExternalInput")
    a2_d = nc.dram_tensor("a2t", [SEG, 128, 2, H2], fmm, kind="ExternalInput")
    b2_d = nc.dram_tensor("b2", [SEG, 128, 2], f32, kind="ExternalInput")
    av_d = nc.dram_tensor("avt", [SEG, 128, 2, QOp], fmm, kind="ExternalInput")
    bv_d = nc.dram_tensor("bv", [SEG, QOp], f32, kind="ExternalInput")
    ones_s_d = nc.dram_tensor("ones_s", [QOp, Qp], fmm, kind="ExternalInput")
    ones_r_d = nc.dram_tensor("ones_r", [Qp, QOp], fmm, kind="ExternalInput")
    out_d = nc.dram_tensor("out", [QO, SEG * Cc], f32, kind="ExternalOutput")

    import contextlib

    lp = (
        nc.allow_low_precision(reason="float32r matmul operands are intentional")
        if use_f32r
        else contextlib.nullcontext()
    )
    with lp, tile.TileContext(nc) as tc:
        with (
            tc.tile_pool(name="xp", bufs=1) as xp,
            tc.tile_pool(name="wp", bufs=2) as wp,
            tc.tile_pool(name="zp", bufs=2) as zp,
            tc.tile_pool(name="op", bufs=3) as op_,
            tc.tile_pool(name="psA", bufs=2, space="PSUM") as psA,
            tc.tile_pool(name="psB", bufs=2, space="PSUM") as psB,
        ):
            ones_s = xp.tile([QOp, Qp], fmm)
            nc.gpsimd.dma_start(ones_s[:], ones_s_d[:])
            ones_r = xp.tile([Qp, QOp], fmm)
            nc.gpsimd.dma_start(ones_r[:], ones_r_d[:])

            for s in range(SEG):
                a1 = wp.tile([D + 1, H1], fmm, tag="a1")
                nc.sync.dma_start(a1[:], a1_d[s])
                xt = xp.tile([D + 1, Cc], fmm, tag=f"xc{s}")
                nc.gpsimd.dma_start(xt[:], xt_d[:, s * Cc : (s + 1) * Cc])
                a2 = wp.tile([128, 2, H2], fmm, tag="a2")
                nc.sync.dma_start(a2[:, 0, :], a2_d[s, :, 0, :])
                nc.sync.dma_start(a2[:, 1, :], a2_d[s, :, 1, :])
                b2 = wp.tile([128, 2], f32, tag="b2")
                nc.sync.dma_start(b2[:], b2_d[s])
                av = wp.tile([128, 2, QOp], fmm, tag="av")
                nc.sync.dma_start(av[:], av_d[s])
                bv = wp.tile([QOp, 1], f32, tag="bv")
                nc.sync.dma_start(bv[:], bv_d[s][:, None])

                z1 = zp.tile([128, 2, Cc], fmm, tag="z1")
                z2 = zp.tile([128, 2, Cc], fmm, tag="z2")

                # ---- layer 1: z1 = relu(xb @ A1eff.T), bias folded in ones col
                for m in range(2):
                    for n in range(NT):
                        p1 = psA.tile([128, 512], f32, tag="l1")
                        nc.tensor.matmul(
                            p1[:],
                            a1[:, m * 128 : (m + 1) * 128],
                            xt[:, n * 512 : (n + 1) * 512],
                            start=True,
                            stop=True,
                        )
                        dst = z1[:, m, n * 512 : (n + 1) * 512]
                        if n % 2 == 0:
                            nc.scalar.activation(dst, p1[:], AF.Relu)
                        else:
                            nc.vector.tensor_scalar(
                                out=dst, in0=p1[:], scalar1=0.0, scalar2=None,
                                op0=ALU.max,
                            )

                # ---- layer 2: z2 = relu(z1b @ A2eff.T + b2)
                for m in range(2):
                    for n in range(NT):
                        p2 = psA.tile([128, 512], f32, tag="l2")
                        for k in range(2):
                            nc.tensor.matmul(
                                p2[:],
                                a2[:, k, m * 128 : (m + 1) * 128],
                                z1[:, k, n * 512 : (n + 1) * 512],
                                start=(k == 0),
                                stop=(k == 1),
                            )
                        dst = z2[:, m, n * 512 : (n + 1) * 512]
                        if n % 2 == 1:
                            nc.scalar.activation(
                                dst, p2[:], AF.Relu, bias=b2[:, m : m + 1]
                            )
                        else:
                            nc.vector.tensor_scalar(
                                out=dst, in0=p2[:],
                                scalar1=b2[:, m : m + 1], scalar2=0.0,
                                op0=ALU.add, op1=ALU.max,
                            )

                # ---- heads + grouped softmax, head-major (QOp x 512) tiles
                for n in range(NT):
                    nsl = slice(n * 512, (n + 1) * 512)
                    ph = psA.tile([QOp, 512], f32, tag="hd")
                    for k in range(2):
                        nc.tensor.matmul(
                            ph[:],
                            av[:, k, :],
                            z2[:, k, nsl],
                            start=(k == 0),
                            stop=(k == 1),
                        )
                    # e = exp(logits + bias); pad row 35 gets bias -1e30 -> 0
                    e = op_.tile([QOp, 512], fmm, tag="e")
                    nc.scalar.activation(e[:], ph[:], AF.Exp, bias=bv[:])
                    # group sums into rows 0..6 of a shared PSUM bank; row Q
                    # holds the total so every row stays positive/finite
                    smrb = psB.tile([QOp, 512], f32, tag="smrb")
                    nc.tensor.matmul(
                        smrb[:Qp, :], ones_s[:], e[:], start=True, stop=True
                    )
                    rt_f = op_.tile([Qp, 512], f32, tag="rt_f")
                    nc.vector.reciprocal_approx_fast(rt_f[:], smrb[:Qp, :])
                    if use_f32r:
                        rt = op_.tile([Qp, 512], fmm, tag="rt")
                        nc.vector.tensor_copy(rt[:], rt_f[:])
                    else:
                        rt = rt_f
                    # broadcast reciprocal back to the 35 head rows
                    nc.tensor.matmul(smrb[:], ones_r[:], rt[:], start=True, stop=True)
                    outm = op_.tile([QO, 512], f32, tag="outm")
                    nc.vector.tensor_tensor(outm[:], e[:QO, :], smrb[:QO, :], ALU.mult)
                    nc.gpsimd.dma_start(out_d[:, s * Cc + n * 512 : s * Cc + (n + 1) * 512], outm[:])

    nc.compile()
    return nc


def _get_program(Cc, use_f32r):
    key = (Cc, use_f32r)
    if key not in _PROG_CACHE:
        _PROG_CACHE[key] = _build_program(Cc, use_f32r)
    return _PROG_CACHE[key]


def kernel(**inputs):
    global LAST_RESULTS
    x = np.ascontiguousarray(np.asarray(inputs["x"], dtype=np.float32))
    ids = np.asarray(inputs["judge_ids"]).astype(np.int64).ravel()
    W1_w = np.asarray(inputs["W1_w"], np.float32)
    W1_b = np.asarray(inputs["W1_b"], np.float32)
    W2_w = np.asarray(inputs["W2_w"], np.float32)
    W2_b = np.asarray(inputs["W2_b"], np.float32)
    W1a_w = np.asarray(inputs["W1a_w"], np.float32)
    W1a_b = np.asarray(inputs["W1a_b"], np.float32)
    W2a_w = np.asarray(inputs["W2a_w"], np.float32)
    W2a_b = np.asarray(inputs["W2a_b"], np.float32)
    V_w = np.asarray(inputs["V_w"], np.float32)
    V_b = np.asarray(inputs["V_b"], np.float32)
    Va_w = np.asarray(inputs["Va_w"], np.float32)
    Va_b = np.asarray(inputs["Va_b"], np.float32)

    Bx = x.shape[0]
    cnts = np.bincount(ids, minlength=J)
    Cc = 1536
    mx = int(cnts.max())
    if 2 * Cc < mx:
        Cc = ((mx + 1) // 2 + 511) // 512 * 512

    # effective per-judge weights (shared + judge-specific, biases folded)
    A1 = (W1_w[None] + W1a_w).copy()                      # (J, H1, D+1)
    A1[:, :, D] += W1_b[None] + W1a_b
    A2 = W2_w[None] + W2a_w                               # (J, H2, H1+1)
    b2 = A2[:, :, H1] + W2_b[None] + W2a_b                # (J, H2)
    A2c = A2[:, :, :H1]                                   # (J, H2, H1)
    AV = (V_w[None] + Va_w).reshape(J, QO, H2 + 1)
    bV = (AV[:, :, H2] + (V_b[None] + Va_b).reshape(J, QO)).astype(np.float32)
    AVc = AV[:, :, :H2]

    # SBUF layouts
    a1sb = np.ascontiguousarray(np.transpose(A1, (0, 2, 1)))  # (J, 36, 256)
    a2sb = np.ascontiguousarray(
        np.transpose(A2c.reshape(J, H2, 2, 128), (0, 3, 2, 1))
    )  # (J, 128, 2, 256): [j,p,k,m] = A2c[j][m, k*128+p]
    b2sb = np.ascontiguousarray(np.transpose(b2.reshape(J, 2, 128), (0, 2, 1)))
    avsb = np.transpose(AVc.reshape(J, QO, 2, 128), (0, 3, 2, 1))  # (J,128,2,35)
    avsb = np.concatenate(
        [avsb, np.zeros((J, 128, 2, 1), np.float32)], axis=3
    )  # pad head out dim to 36
    avsb = np.ascontiguousarray(avsb)
    bvp = np.concatenate(
        [bV, np.full((J, 1), -1e30, np.float32)], axis=1
    )  # (J, 36): pad row bias -> exp = 0

    # block-ones matrices for the softmax group sum / reciprocal broadcast
    ones_s = np.zeros((QOp, Qp), np.float32)
    for o_ in range(QO):
        ones_s[o_, o_ // O] = 1.0
    ones_s[:, Q] = 1.0          # row Q of sums = total (keeps recip finite)
    ones_r = np.zeros((Qp, QOp), np.float32)
    for o_ in range(QO):
        ones_r[o_ // O, o_] = 1.0

    # slot -> sample map: judge j owns slots [j*2Cc, (j+1)*2Cc)
    order = np.argsort(ids, kind="stable")
    slot2samp = np.full(NCHUNKS * Cc, -1, np.int64)
    pos = 0
    for j in range(J):
        k = int(cnts[j])
        slot2samp[j * 2 * Cc : j * 2 * Cc + k] = order[pos : pos + k]
        pos += k
    chunk_judge = np.repeat(np.arange(J), 2)

    in_maps = []
    core_meta = []
    for c in range(NCORES):
        sl = slot2samp[c * SEG * Cc : (c + 1) * SEG * Cc]
        valid = sl >= 0
        Xc = np.zeros((SEG * Cc, D + 1), np.float32)
        Xc[valid, :D] = x[sl[valid]]
        Xc[:, D] = 1.0
        js = chunk_judge[c * SEG : (c + 1) * SEG]
        in_maps.append(
            {
                "xt": np.ascontiguousarray(Xc.T),
                "a1t": np.ascontiguousarray(a1sb[js]),
                "a2t": np.ascontiguousarray(a2sb[js]),
                "b2": np.ascontiguousarray(b2sb[js]),
                "avt": np.ascontiguousarray(avsb[js]),
                "bv": np.ascontiguousarray(bvp[js]),
                "ones_s": ones_s,
                "ones_r": ones_r,
            }
        )
        core_meta.append((sl, valid))

    nc = _get_program(Cc, USE_F32R)
    from concourse.bass_utils import run_bass_kernel_spmd

    res = run_bass_kernel_spmd(
        nc,
        in_maps,
        core_ids=list(range(NCORES)),
        trace=TRACE,
    )
    LAST_RESULTS = res

    full = np.zeros((Bx, Q, O), np.float32)
    for c in range(NCORES):
        oc = np.asarray(res.results[c]["out"]).T    # (SEG*Cc, 35)
        sl, valid = core_meta[c]
        full[sl[valid]] = oc[valid].reshape(-1, Q, O)
    return full


# revision 19
# speedup vs baseline: 1.1940x; 1.0289x over previous
"""Trainium2 Bass kernel for nn_CalibrationNetwork (MoE routing over 12 judges).

Strategy: shared + judge-specific weights are pre-summed on the host into 12
effective per-judge MLPs (the einsum+take_along_axis in the reference is just
"route each sample through the MLP of its judge").  Samples are sorted by
judge id on the host, each judge's slots padded to a fixed capacity 2*Cc, and
the resulting 24 fixed-size chunks (2 per judge) are dealt 3-per-core to the 8
NeuronCores.  Every core runs the same static Bass/Tile program: for each of
its 3 chunks, dense matmuls (layer1 K=36, layer2 K=256, heads K=256) with
relu/bias fused into the PSUM-evacuation.  The 7x5 per-question softmax runs
in head-major layout: exp(logits+bias) via the ACT engine's per-partition
bias, group sums and the reciprocal broadcast via exact block-ones matmuls on
the PE.  Output is written head-major (contiguous rows) and transposed back
on the host during the unshard scatter.
"""

import os
import sys

import numpy as np

for _p in ("/opt/trn_rl_repo", "/root/.axon_site/_ro/trn_rl_repo"):
    if os.path.isdir(_p) and _p not in sys.path:
        sys.path.insert(0, _p)

B, D, H1, H2, J, Q, O = 32768, 35, 256, 256, 12, 7, 5
NCORES = 8
SEG = 3                    # chunks per core
NCHUNKS = NCORES * SEG     # 24 = 2 chunks per judge
QO = Q * O                 # 35
QOp = QO + 1               # padded head dim (f32r needs even sizes)
Qp = Q + 1                 # padded question dim

USE_F32R = True            # PE fast-fp32 mode (1 cyc/row vs 4 for fp32)
TRACE = False              # set True in test harness to collect NTFF profile
LAST_RESULTS = None        # BassKernelResults of the last run (for test.py)

_PROG_CACHE = {}


def _build_program(Cc, use_f32r):
    import concourse.bass as bass
    import concourse.tile as tile
    from concourse import bacc, mybir

    f32 = mybir.dt.float32
    fmm = mybir.dt.float32r if use_f32r else f32
    AF = mybir.ActivationFunctionType
    ALU = mybir.AluOpType

    NT = Cc // 512            # 512-wide n-tiles per chunk

    nc = bacc.Bacc(None, target_bir_lowering=False, debug=False, num_swdge_queues=4)

    xt_d = nc.dram_tensor("xt", [D + 1, SEG * Cc], fmm, kind="ExternalInput")
    a1_d = nc.dram_tensor("a1t", [SEG, D + 1, H1], fmm, kind="ExternalInput")
    a2_d = nc.dram_tensor("a2t", [SEG, 128, 2, H2], fmm, kind="ExternalInput")
    b2_d = nc.dram_tensor("b2", [SEG, 128, 2], f32, kind="ExternalInput")
    av_d = nc.dram_tensor("avt", [SEG, 128, 2, QOp], fmm, kind="ExternalInput")
    bv_d = nc.dram_tensor("bv", [SEG, QOp], f32, kind="ExternalInput")
    ones_s_d = nc.dram_tensor("ones_s", [QOp, Qp], fmm, kind="ExternalInput")
    ones_r_d = nc.dram_tensor("ones_r", [Qp, QOp], fmm, kind="ExternalInput")
    out_d = nc.dram_tensor("out", [QO, SEG * Cc], f32, kind="ExternalOutput")

    import contextlib

    lp = (
        nc.allow_low_precision(reason="float32r matmul operands are intentional")
        if use_f32r
        else contextlib.nullcontext()
    )
    with lp, tile.TileContext(nc) as tc:
        with (
            tc.tile_pool(name="xp", bufs=1) as xp,
            tc.tile_pool(name="wp", bufs=2) as wp,
            tc.tile_pool(name="zp", bufs=2) as zp,
            tc.tile_pool(name="op", bufs=3) as op_,
            tc.tile_pool(name="psA", bufs=2, space="PSUM") as psA,
            tc.tile_pool(name="psB", bufs=2, space="PSUM") as psB,
        ):
            ones_s = xp.tile([QOp, Qp], fmm)
            nc.gpsimd.dma_start(ones_s[:], ones_s_d[:])
            ones_r = xp.tile([Qp, QOp], fmm)
            nc.gpsimd.dma_start(ones_r[:], ones_r_d[:])

            for s in range(SEG):
                a1 = wp.tile([D + 1, H1], fmm, tag="a1")
                nc.sync.dma_start(a1[:], a1_d[s])
                xt = xp.tile([D + 1, Cc], fmm, tag=f"xc{s}")
                nc.gpsimd.dma_start(xt[:], xt_d[:, s * Cc : (s + 1) * Cc])
                a2 = wp.tile([128, 2, H2], fmm, tag="a2")
                nc.sync.dma_start(a2[:, 0, :], a2_d[s, :, 0, :])
                nc.scalar.dma_start(a2[:, 1, :], a2_d[s, :, 1, :])
                b2 = wp.tile([128, 2], f32, tag="b2")
                nc.sync.dma_start(b2[:], b2_d[s])
                av = wp.tile([128, 2, QOp], fmm, tag="av")
                nc.scalar.dma_start(av[:], av_d[s])
                bv = wp.tile([QOp, 1], f32, tag="bv")
                nc.scalar.dma_start(bv[:], bv_d[s][:, None])

                z1 = zp.tile([128, 2, Cc], fmm, tag="z1")
                z2 = zp.tile([128, 2, Cc], fmm, tag="z2")

                # ---- layer 1: z1 = relu(xb @ A1eff.T), bias folded in ones col
                for m in range(2):
                    for n in range(NT):
                        p1 = psA.tile([128, 512], f32, tag="l1")
                        nc.tensor.matmul(
                            p1[:],
                            a1[:, m * 128 : (m + 1) * 128],
                            xt[:, n * 512 : (n + 1) * 512],
                            start=True,
                            stop=True,
                        )
                        dst = z1[:, m, n * 512 : (n + 1) * 512]
                        if n % 2 == 0:
                            nc.scalar.activation(dst, p1[:], AF.Relu)
                        else:
                            nc.vector.tensor_scalar(
                                out=dst, in0=p1[:], scalar1=0.0, scalar2=None,
                                op0=ALU.max,
                            )

                # ---- layer 2: z2 = relu(z1b @ A2eff.T + b2)
                for m in range(2):
                    for n in range(NT):
                        p2 = psA.tile([128, 512], f32, tag="l2")
                        for k in range(2):
                            nc.tensor.matmul(
                                p2[:],
                                a2[:, k, m * 128 : (m + 1) * 128],
                                z1[:, k, n * 512 : (n + 1) * 512],
                                start=(k == 0),
                                stop=(k == 1),
                            )
                        dst = z2[:, m, n * 512 : (n + 1) * 512]
                        if n % 2 == 1:
                            nc.scalar.activation(
                                dst, p2[:], AF.Relu, bias=b2[:, m : m + 1]
                            )
                        else:
                            nc.vector.tensor_scalar(
                                out=dst, in0=p2[:],
                                scalar1=b2[:, m : m + 1], scalar2=0.0,
                                op0=ALU.add, op1=ALU.max,
                            )

                # ---- heads + grouped softmax, head-major (QOp x 512) tiles
                for n in range(NT):
                    nsl = slice(n * 512, (n + 1) * 512)
                    ph = psA.tile([QOp, 512], f32, tag="hd")
                    for k in range(2):
                        nc.tensor.matmul(
                            ph[:],
                            av[:, k, :],
                            z2[:, k, nsl],
                            start=(k == 0),
                            stop=(k == 1),
                        )
                    # e = exp(logits + bias); pad row 35 gets bias -1e30 -> 0
                    e = op_.tile([QOp, 512], fmm, tag="e")
                    nc.scalar.activation(e[:], ph[:], AF.Exp, bias=bv[:])
                    # group sums into rows 0..6 of a shared PSUM bank; row Q
                    # holds the total so every row stays positive/finite
                    smrb = psB.tile([QOp, 512], f32, tag="smrb")
                    nc.tensor.matmul(
                        smrb[:Qp, :], ones_s[:], e[:], start=True, stop=True
                    )
                    rt_f = op_.tile([Qp, 512], f32, tag="rt_f")
                    nc.vector.reciprocal_approx_fast(rt_f[:], smrb[:Qp, :])
                    if use_f32r:
                        rt = op_.tile([Qp, 512], fmm, tag="rt")
                        nc.vector.tensor_copy(rt[:], rt_f[:])
                    else:
                        rt = rt_f
                    # broadcast reciprocal back to the 35 head rows
                    nc.tensor.matmul(smrb[:], ones_r[:], rt[:], start=True, stop=True)
                    outm = op_.tile([QO, 512], f32, tag="outm")
                    nc.vector.tensor_tensor(outm[:], e[:QO, :], smrb[:QO, :], ALU.mult)
                    oeng = [nc.gpsimd, nc.sync, nc.scalar][(s * NT + n) % 3]
                    oeng.dma_start(out_d[:, s * Cc + n * 512 : s * Cc + (n + 1) * 512], outm[:])

    nc.compile()
    return nc


def _get_program(Cc, use_f32r):
    key = (Cc, use_f32r)
    if key not in _PROG_CACHE:
        _PROG_CACHE[key] = _build_program(Cc, use_f32r)
    return _PROG_CACHE[key]


def kernel(**inputs):
    global LAST_RESULTS
    x = np.ascontiguousarray(np.asarray(inputs["x"], dtype=np.float32))
    ids = np.asarray(inputs["judge_ids"]).astype(np.int64).ravel()
    W1_w = np.asarray(inputs["W1_w"], np.float32)
    W1_b = np.asarray(inputs["W1_b"], np.float32)
    W2_w = np.asarray(inputs["W2_w"], np.float32)
    W2_b = np.asarray(inputs["W2_b"], np.float32)
    W1a_w = np.asarray(inputs["W1a_w"], np.float32)
    W1a_b = np.asarray(inputs["W1a_b"], np.float32)
    W2a_w = np.asarray(inputs["W2a_w"], np.float32)
    W2a_b = np.asarray(inputs["W2a_b"], np.float32)
    V_w = np.asarray(inputs["V_w"], np.float32)
    V_b = np.asarray(inputs["V_b"], np.float32)
    Va_w = np.asarray(inputs["Va_w"], np.float32)
    Va_b = np.asarray(inputs["Va_b"], np.float32)

    Bx = x.shape[0]
    cnts = np.bincount(ids, minlength=J)
    Cc = 1536
    mx = int(cnts.max())
    if 2 * Cc < mx:
        Cc = ((mx + 1) // 2 + 511) // 512 * 512

    # effective per-judge weights (shared + judge-specific, biases folded)
    A1 = (W1_w[None] + W1a_w).copy()                      # (J, H1, D+1)
    A1[:, :, D] += W1_b[None] + W1a_b
    A2 = W2_w[None] + W2a_w                               # (J, H2, H1+1)
    b2 = A2[:, :, H1] + W2_b[None] + W2a_b                # (J, H2)
    A2c = A2[:, :, :H1]                                   # (J, H2, H1)
    AV = (V_w[None] + Va_w).reshape(J, QO, H2 + 1)
    bV = (AV[:, :, H2] + (V_b[None] + Va_b).reshape(J, QO)).astype(np.float32)
    AVc = AV[:, :, :H2]

    # SBUF layouts
    a1sb = np.ascontiguousarray(np.transpose(A1, (0, 2, 1)))  # (J, 36, 256)
    a2sb = np.ascontiguousarray(
        np.transpose(A2c.reshape(J, H2, 2, 128), (0, 3, 2, 1))
    )  # (J, 128, 2, 256): [j,p,k,m] = A2c[j][m, k*128+p]
    b2sb = np.ascontiguousarray(np.transpose(b2.reshape(J, 2, 128), (0, 2, 1)))
    avsb = np.transpose(AVc.reshape(J, QO, 2, 128), (0, 3, 2, 1))  # (J,128,2,35)
    avsb = np.concatenate(
        [avsb, np.zeros((J, 128, 2, 1), np.float32)], axis=3
    )  # pad head out dim to 36
    avsb = np.ascontiguousarray(avsb)
    bvp = np.concatenate(
        [bV, np.full((J, 1), -1e30, np.float32)], axis=1
    )  # (J, 36): pad row bias -> exp = 0

    # block-ones matrices for the softmax group sum / reciprocal broadcast
    ones_s = np.zeros((QOp, Qp), np.float32)
    for o_ in range(QO):
        ones_s[o_, o_ // O] = 1.0
    ones_s[:, Q] = 1.0          # row Q of sums = total (keeps recip finite)
    ones_r = np.zeros((Qp, QOp), np.float32)
    for o_ in range(QO):
        ones_r[o_ // O, o_] = 1.0

    # slot -> sample map: judge j owns slots [j*2Cc, (j+1)*2Cc)
    order = np.argsort(ids, kind="stable")
    slot2samp = np.full(NCHUNKS * Cc, -1, np.int64)
    pos = 0
    for j in range(J):
        k = int(cnts[j])
        slot2samp[j * 2 * Cc : j * 2 * Cc + k] = order[pos : pos + k]
        pos += k
    chunk_judge = np.repeat(np.arange(J), 2)

    in_maps = []
    core_meta = []
    for c in range(NCORES):
        sl = slot2samp[c * SEG * Cc : (c + 1) * SEG * Cc]
        valid = sl >= 0
        Xc = np.zeros((SEG * Cc, D + 1), np.float32)
        Xc[valid, :D] = x[sl[valid]]
        Xc[:, D] = 1.0
        js = chunk_judge[c * SEG : (c + 1) * SEG]
        in_maps.append(
            {
                "xt": np.ascontiguousarray(Xc.T),
                "a1t": np.ascontiguousarray(a1sb[js]),
                "a2t": np.ascontiguousarray(a2sb[js]),
                "b2": np.ascontiguousarray(b2sb[js]),
                "avt": np.ascontiguousarray(avsb[js]),
                "bv": np.ascontiguousarray(bvp[js]),
                "ones_s": ones_s,
                "ones_r": ones_r,
            }
        )
        core_meta.append((sl, valid))

    nc = _get_program(Cc, USE_F32R)
    from concourse.bass_utils import run_bass_kernel_spmd

    res = run_bass_kernel_spmd(
        nc,
        in_maps,
        core_ids=list(range(NCORES)),
        trace=TRACE,
    )
    LAST_RESULTS = res

    full = np.zeros((Bx, Q, O), np.float32)
    for c in range(NCORES):
        oc = np.asarray(res.results[c]["out"]).T    # (SEG*Cc, 35)
        sl, valid = core_meta[c]
        full[sl[valid]] = oc[valid].reshape(-1, Q, O)
    return full


# revision 22
# speedup vs baseline: 1.3939x; 1.1675x over previous
"""Trainium2 Bass kernel for nn_CalibrationNetwork (MoE routing over 12 judges).

Strategy: shared + judge-specific weights are pre-summed on the host into 12
effective per-judge MLPs (the einsum+take_along_axis in the reference is just
"route each sample through the MLP of its judge").  Samples are sorted by
judge id on the host, each judge's slots padded to a fixed capacity 2*Cc, and
the resulting 24 fixed-size chunks (2 per judge) are dealt 3-per-core to the 8
NeuronCores.  Every core runs the same static Bass/Tile program: for each of
its 3 chunks, dense matmuls (layer1 K=36, layer2 K=256, heads K=256) with
relu/bias fused into the PSUM-evacuation.  The 7x5 per-question softmax runs
in head-major layout: exp(logits+bias) via the ACT engine's per-partition
bias, group sums and the reciprocal broadcast via exact block-ones matmuls on
the PE.  Output is written head-major (contiguous rows) and transposed back
on the host during the unshard scatter.
"""

import os
import sys

import numpy as np

for _p in ("/opt/trn_rl_repo", "/root/.axon_site/_ro/trn_rl_repo"):
    if os.path.isdir(_p) and _p not in sys.path:
        sys.path.insert(0, _p)

B, D, H1, H2, J, Q, O = 32768, 35, 256, 256, 12, 7, 5
NCORES = 8
SEG = 3                    # chunks per core
NCHUNKS = NCORES * SEG     # 24 = 2 chunks per judge
QO = Q * O                 # 35
QOp = QO + 1               # padded head dim (f32r needs even sizes)
Qp = Q + 1                 # padded question dim

USE_F32R = True            # PE fast-fp32 mode (1 cyc/row vs 4 for fp32)
TRACE = False              # set True in test harness to collect NTFF profile
LAST_RESULTS = None        # BassKernelResults of the last run (for test.py)

_PROG_CACHE = {}


def _build_program(Cc, use_f32r):
    import concourse.bass as bass
    import concourse.tile as tile
    from concourse import bacc, mybir

    f32 = mybir.dt.float32
    fmm = mybir.dt.float32r if use_f32r else f32
    AF = mybir.ActivationFunctionType
    ALU = mybir.AluOpType

    NT = Cc // 512            # 512-wide n-tiles per chunk

    nc = bacc.Bacc(None, target_bir_lowering=False, debug=False, num_swdge_queues=4)

    xt_d = nc.dram_tensor("xt", [D + 1, SEG * Cc], fmm, kind="ExternalInput")
    a1_d = nc.dram_tensor("a1t", [SEG, D + 1, H1], fmm, kind="ExternalInput")
    a2_d = nc.dram_tensor("a2t", [SEG, 128, 2, H2], fmm, kind="ExternalInput")
    b2_d = nc.dram_tensor("b2", [SEG, 128, 2], f32, kind="ExternalInput")
    av_d = nc.dram_tensor("avt", [SEG, 128, 2, QOp], fmm, kind="ExternalInput")
    bv_d = nc.dram_tensor("bv", [SEG, QOp], f32, kind="ExternalInput")
    ones_s_d = nc.dram_tensor("ones_s", [QOp, Qp], fmm, kind="ExternalInput")
    ones_r_d = nc.dram_tensor("ones_r", [Qp, QOp], fmm, kind="ExternalInput")
    out_d = nc.dram_tensor("out", [QO, SEG * Cc], f32, kind="ExternalOutput")

    import contextlib

    lp = (
        nc.allow_low_precision(reason="float32r matmul operands are intentional")
        if use_f32r
        else contextlib.nullcontext()
    )
    with lp, tile.TileContext(nc) as tc:
        with (
            tc.tile_pool(name="xp", bufs=1) as xp,
            tc.tile_pool(name="wp", bufs=2) as wp,
            tc.tile_pool(name="zp", bufs=2) as zp,
            tc.tile_pool(name="op", bufs=3) as op_,
            tc.tile_pool(name="psA", bufs=2, space="PSUM") as psA,
            tc.tile_pool(name="psB", bufs=2, space="PSUM") as psB,
        ):
            ones_s = xp.tile([QOp, Qp], fmm)
            nc.gpsimd.dma_start(ones_s[:], ones_s_d[:])
            ones_r = xp.tile([Qp, QOp], fmm)
            nc.gpsimd.dma_start(ones_r[:], ones_r_d[:])

            # PE warmup: dummy matmuls during the initial DMA wait keep the
            # HAM clock-gate busy so real matmuls start at full clock
            wsrc = xp.tile([128, 512], f32, tag="warmsrc")
            nc.vector.memset(wsrc[:], 0.0)
            wtile = xp.tile([128, 512], fmm, tag="warm")
            nc.vector.tensor_copy(wtile[:], wsrc[:])
            wps = psA.tile([128, 512], f32, tag="hd")
            for _ in range(12):
                nc.tensor.matmul(
                    wps[:], wtile[:, :128], wtile[:], start=True, stop=True
                )

            for s in range(SEG):
                a1 = wp.tile([D + 1, H1], fmm, tag="a1")
                nc.sync.dma_start(a1[:], a1_d[s])
                xt = xp.tile([D + 1, Cc], fmm, tag=f"xc{s}")
                nc.gpsimd.dma_start(xt[:], xt_d[:, s * Cc : (s + 1) * Cc])
                a2 = wp.tile([128, 2, H2], fmm, tag="a2")
                nc.sync.dma_start(a2[:, 0, :], a2_d[s, :, 0, :])
                nc.scalar.dma_start(a2[:, 1, :], a2_d[s, :, 1, :])
                b2 = wp.tile([128, 2], f32, tag="b2")
                nc.sync.dma_start(b2[:], b2_d[s])
                av = wp.tile([128, 2, QOp], fmm, tag="av")
                nc.scalar.dma_start(av[:], av_d[s])
                bv = wp.tile([QOp, 1], f32, tag="bv")
                nc.scalar.dma_start(bv[:], bv_d[s][:, None])

                z1 = zp.tile([128, 2, Cc], fmm, tag="z1")
                z2 = zp.tile([128, 2, Cc], fmm, tag="z2")

                # ---- layer 1: z1 = relu(xb @ A1eff.T), bias folded in ones col
                for m in range(2):
                    for n in range(NT):
                        p1 = psA.tile([128, 512], f32, tag="l1")
                        nc.tensor.matmul(
                            p1[:],
                            a1[:, m * 128 : (m + 1) * 128],
                            xt[:, n * 512 : (n + 1) * 512],
                            start=True,
                            stop=True,
                        )
                        dst = z1[:, m, n * 512 : (n + 1) * 512]
                        if n % 2 == 0:
                            nc.scalar.activation(dst, p1[:], AF.Relu)
                        else:
                            nc.vector.tensor_scalar(
                                out=dst, in0=p1[:], scalar1=0.0, scalar2=None,
                                op0=ALU.max,
                            )

                # ---- layer 2: z2 = relu(z1b @ A2eff.T + b2)
                for m in range(2):
                    for n in range(NT):
                        p2 = psA.tile([128, 512], f32, tag="l2")
                        for k in range(2):
                            nc.tensor.matmul(
                                p2[:],
                                a2[:, k, m * 128 : (m + 1) * 128],
                                z1[:, k, n * 512 : (n + 1) * 512],
                                start=(k == 0),
                                stop=(k == 1),
                            )
                        dst = z2[:, m, n * 512 : (n + 1) * 512]
                        if n % 2 == 1:
                            nc.scalar.activation(
                                dst, p2[:], AF.Relu, bias=b2[:, m : m + 1]
                            )
                        else:
                            nc.vector.tensor_scalar(
                                out=dst, in0=p2[:],
                                scalar1=b2[:, m : m + 1], scalar2=0.0,
                                op0=ALU.add, op1=ALU.max,
                            )

                # ---- heads + grouped softmax, head-major (QOp x 512) tiles
                for n in range(NT):
                    nsl = slice(n * 512, (n + 1) * 512)
                    ph = psA.tile([QOp, 512], f32, tag="hd")
                    for k in range(2):
                        nc.tensor.matmul(
                            ph[:],
                            av[:, k, :],
                            z2[:, k, nsl],
                            start=(k == 0),
                            stop=(k == 1),
                        )
                    # e = exp(logits + bias); pad row 35 gets bias -1e30 -> 0
                    e = op_.tile([QOp, 512], fmm, tag="e")
                    nc.scalar.activation(e[:], ph[:], AF.Exp, bias=bv[:])
                    # group sums into rows 0..6 of a shared PSUM bank; row Q
                    # holds the total so every row stays positive/finite
                    smrb = psB.tile([QOp, 512], f32, tag="smrb")
                    nc.tensor.matmul(
                        smrb[:Qp, :], ones_s[:], e[:], start=True, stop=True
                    )
                    rt_f = op_.tile([Qp, 512], f32, tag="rt_f")
                    nc.vector.reciprocal_approx_fast(rt_f[:], smrb[:Qp, :])
                    if use_f32r:
                        rt = op_.tile([Qp, 512], fmm, tag="rt")
                        nc.vector.tensor_copy(rt[:], rt_f[:])
                    else:
                        rt = rt_f
                    # broadcast reciprocal back to the 35 head rows
                    nc.tensor.matmul(smrb[:], ones_r[:], rt[:], start=True, stop=True)
                    outm = op_.tile([QO, 512], f32, tag="outm")
                    nc.vector.tensor_tensor(outm[:], e[:QO, :], smrb[:QO, :], ALU.mult)
                    oeng = [nc.gpsimd, nc.sync, nc.scalar][(s * NT + n) % 3]
                    oeng.dma_start(out_d[:, s * Cc + n * 512 : s * Cc + (n + 1) * 512], outm[:])

    nc.compile()
    return nc


def _get_program(Cc, use_f32r):
    key = (Cc, use_f32r)
    if key not in _PROG_CACHE:
        _PROG_CACHE[key] = _build_program(Cc, use_f32r)
    return _PROG_CACHE[key]


def kernel(**inputs):
    global LAST_RESULTS
    x = np.ascontiguousarray(np.asarray(inputs["x"], dtype=np.float32))
    ids = np.asarray(inputs["judge_ids"]).astype(np.int64).ravel()
    W1_w = np.asarray(inputs["W1_w"], np.float32)
    W1_b = np.asarray(inputs["W1_b"], np.float32)
    W2_w = np.asarray(inputs["W2_w"], np.float32)
    W2_b = np.asarray(inputs["W2_b"], np.float32)
    W1a_w = np.asarray(inputs["W1a_w"], np.float32)
    W1a_b = np.asarray(inputs["W1a_b"], np.float32)
    W2a_w = np.asarray(inputs["W2a_w"], np.float32)
    W2a_b = np.asarray(inputs["W2a_b"], np.float32)
    V_w = np.asarray(inputs["V_w"], np.float32)
    V_b = np.asarray(inputs["V_b"], np.float32)
    Va_w = np.asarray(inputs["Va_w"], np.float32)
    Va_b = np.asarray(inputs["Va_b"], np.float32)

    Bx = x.shape[0]
    cnts = np.bincount(ids, minlength=J)
    Cc = 1536
    mx = int(cnts.max())
    if 2 * Cc < mx:
        Cc = ((mx + 1) // 2 + 511) // 512 * 512

    # effective per-judge weights (shared + judge-specific, biases folded)
    A1 = (W1_w[None] + W1a_w).copy()                      # (J, H1, D+1)
    A1[:, :, D] += W1_b[None] + W1a_b
    A2 = W2_w[None] + W2a_w                               # (J, H2, H1+1)
    b2 = A2[:, :, H1] + W2_b[None] + W2a_b                # (J, H2)
    A2c = A2[:, :, :H1]                                   # (J, H2, H1)
    AV = (V_w[None] + Va_w).reshape(J, QO, H2 + 1)
    bV = (AV[:, :, H2] + (V_b[None] + Va_b).reshape(J, QO)).astype(np.float32)
    AVc = AV[:, :, :H2]

    # SBUF layouts
    a1sb = np.ascontiguousarray(np.transpose(A1, (0, 2, 1)))  # (J, 36, 256)
    a2sb = np.ascontiguousarray(
        np.transpose(A2c.reshape(J, H2, 2, 128), (0, 3, 2, 1))
    )  # (J, 128, 2, 256): [j,p,k,m] = A2c[j][m, k*128+p]
    b2sb = np.ascontiguousarray(np.transpose(b2.reshape(J, 2, 128), (0, 2, 1)))
    avsb = np.transpose(AVc.reshape(J, QO, 2, 128), (0, 3, 2, 1))  # (J,128,2,35)
    avsb = np.concatenate(
        [avsb, np.zeros((J, 128, 2, 1), np.float32)], axis=3
    )  # pad head out dim to 36
    avsb = np.ascontiguousarray(avsb)
    bvp = np.concatenate(
        [bV, np.full((J, 1), -1e30, np.float32)], axis=1
    )  # (J, 36): pad row bias -> exp = 0

    # block-ones matrices for the softmax group sum / reciprocal broadcast
    ones_s = np.zeros((QOp, Qp), np.float32)
    for o_ in range(QO):
        ones_s[o_, o_ // O] = 1.0
    ones_s[:, Q] = 1.0          # row Q of sums = total (keeps recip finite)
    ones_r = np.zeros((Qp, QOp), np.float32)
    for o_ in range(QO):
        ones_r[o_ // O, o_] = 1.0

    # slot -> sample map: judge j owns slots [j*2Cc, (j+1)*2Cc)
    order = np.argsort(ids, kind="stable")
    slot2samp = np.full(NCHUNKS * Cc, -1, np.int64)
    pos = 0
    for j in range(J):
        k = int(cnts[j])
        slot2samp[j * 2 * Cc : j * 2 * Cc + k] = order[pos : pos + k]
        pos += k
    chunk_judge = np.repeat(np.arange(J), 2)

    in_maps = []
    core_meta = []
    for c in range(NCORES):
        sl = slot2samp[c * SEG * Cc : (c + 1) * SEG * Cc]
        valid = sl >= 0
        Xc = np.zeros((SEG * Cc, D + 1), np.float32)
        Xc[valid, :D] = x[sl[valid]]
        Xc[:, D] = 1.0
        js = chunk_judge[c * SEG : (c + 1) * SEG]
        in_maps.append(
            {
                "xt": np.ascontiguousarray(Xc.T),
                "a1t": np.ascontiguousarray(a1sb[js]),
                "a2t": np.ascontiguousarray(a2sb[js]),
                "b2": np.ascontiguousarray(b2sb[js]),
                "avt": np.ascontiguousarray(avsb[js]),
                "bv": np.ascontiguousarray(bvp[js]),
                "ones_s": ones_s,
                "ones_r": ones_r,
            }
        )
        core_meta.append((sl, valid))

    nc = _get_program(Cc, USE_F32R)
    from concourse.bass_utils import run_bass_kernel_spmd

    res = run_bass_kernel_spmd(
        nc,
        in_maps,
        core_ids=list(range(NCORES)),
        trace=TRACE,
    )
    LAST_RESULTS = res

    full = np.zeros((Bx, Q, O), np.float32)
    for c in range(NCORES):
        oc = np.asarray(res.results[c]["out"]).T    # (SEG*Cc, 35)
        sl, valid = core_meta[c]
        full[sl[valid]] = oc[valid].reshape(-1, Q, O)
    return full
